# revision 13
# baseline (speedup 1.0000x reference)
"""Trainium2 Bass kernel for nn_MemoryWriter (scatter_memory).

Math (see reference):
    w        = where(gate > 0.01, gate * 0.1, 0)            [B]
    contrib  (q_a, v_a, w_a) scattered to slots top_indices[a, :]
    upd_k[s] = sum_j w_j q_j / (counts>0 ? counts : 1), counts = sum_j w_j
    out_k    = mem_k + 0.9 * mom_k + (1 - 0.9) * upd_k      (mom is zeros)

Because upd is a ratio, the 0.1 UPDATE_RATE cancels between numerator and
denominator; we use raw gated gate values g = gate * (gate > 0.01) as weights
and apply the single (1 - momentum) factor at the end.

Sharding: slot dimension across 8 cores (8192 slots each).  The host performs
the contribution routing that the all-to-all performs in a real distributed
setting (the sharding hint: "route each (query, slot_idx) contribution to the
owning device (all-to-all on flattened top_indices)"): each core receives a
dense buffer of its routed contribution rows, packed [q | v | 1], grouped by
128-slot tile and padded to 128-row blocks.  The device then, per slot tile:
  - builds a weighted one-hot lhsT on the fly: (iota == s) * w, with s = -1
    sentinel on padding rows,
  - one PE float32r matmul per (tile, block) incidence accumulates
    [K-upd | V-upd | counts] into a per-tile PSUM slice,
  - the ACT engine scales by (1-momentum)/counts, and DVE/GpSimd add the
    memory-table tile.
"""

import numpy as np

# ---- problem constants (hardcoded per contest contract) --------------------
N_SLOTS = 65536
DIM = 128
B = 4096
K = 8
NCORES = 8
SPC = N_SLOTS // NCORES      # slots per core = 8192
NT = SPC // 128              # slot tiles per core = 64
P = 128
EL = 320                     # packed row: [q(128) | v(128) | 1 | pad] f32
GATE_THRESH = 0.01
MOMENTUM = 0.9
UPD = float(np.float32(1.0) - np.float32(MOMENTUM))  # exactly as fp32 computes it
USE_F32R = False              # float32r matmul (1 cyc/row at N>=256)

_BUILD_CACHE = {}


def build_nc(nblk: int, incid: tuple):
    """Build the per-core Bass program.

    nblk: number of 128-row contribution blocks.
    incid: per slot-tile tuple of (col, blk, start, stop) incidences; col
    indexes the sv table column holding that incidence's slot-in-tile values.
    """
    import concourse.bacc as bacc
    import concourse.tile as tile
    from concourse import mybir
    from contextlib import ExitStack

    f32 = mybir.dt.float32
    f32r = mybir.dt.float32r
    Alu = mybir.AluOpType
    Act = mybir.ActivationFunctionType

    NCOL = sum(len(v) for v in incid)

    nc = bacc.Bacc("TRN2", target_bir_lowering=False, debug=False)

    mem_k = nc.dram_tensor("mem_k", [SPC, DIM], f32, kind="ExternalInput")
    mem_v = nc.dram_tensor("mem_v", [SPC, DIM], f32, kind="ExternalInput")
    mmdt = f32r if USE_F32R else f32
    routed = nc.dram_tensor("routed", [nblk * 128, EL], mmdt, kind="ExternalInput")
    sv = nc.dram_tensor("sv", [P, NCOL], f32, kind="ExternalInput")
    wb = nc.dram_tensor("wb", [P, nblk], f32, kind="ExternalInput")
    out_k = nc.dram_tensor("out_k", [SPC, DIM], f32, kind="ExternalOutput")
    out_v = nc.dram_tensor("out_v", [SPC, DIM], f32, kind="ExternalOutput")

    G = 8                    # slot tiles per DMA group (512KB per table)
    PG = 4                   # slot tiles per PSUM group (4 banks)

    with tile.TileContext(nc) as tc, ExitStack() as ctx:
        const = ctx.enter_context(tc.tile_pool(name="const", bufs=1))
        gpool = ctx.enter_context(tc.tile_pool(name="gath", bufs=1))
        mpool = ctx.enter_context(tc.tile_pool(name="mem", bufs=3))
        opool = ctx.enter_context(tc.tile_pool(name="outp", bufs=3))
        wpool = ctx.enter_context(tc.tile_pool(name="work", bufs=8))
        spool = ctx.enter_context(tc.tile_pool(name="small", bufs=8))
        upool = ctx.enter_context(tc.tile_pool(name="upd", bufs=4))
        pspool = ctx.enter_context(tc.tile_pool(name="ps", bufs=2, space="PSUM"))

        # constants / routing metadata
        iota_t = const.tile([P, 128], f32)
        nc.gpsimd.iota(
            iota_t[:], pattern=[[1, 128]], channel_multiplier=0,
            allow_small_or_imprecise_dtypes=True,
        )
        sv_t = const.tile([P, NCOL], f32)
        nc.sync.dma_start(sv_t[:], sv[:, :])
        wb_t = const.tile([P, nblk], f32)
        nc.sync.dma_start(wb_t[:], wb[:, :])

        # w = gate * (gate > 0.01), per block column
        msk_t = const.tile([P, nblk], f32)
        nc.vector.tensor_scalar(msk_t[:], wb_t[:], GATE_THRESH, None, op0=Alu.is_gt)
        w_t = const.tile([P, nblk], f32)
        nc.vector.tensor_tensor(w_t[:], wb_t[:], msk_t[:], op=Alu.mult)

        # routed contribution rows: row j at [j%128, j//128, :].  Load in
        # chunks, alternating HWDGE queues (sync / scalar).
        gqv = gpool.tile([P, nblk * EL], mmdt)
        gqv3 = gqv[:].rearrange("p (b e) -> p b e", e=EL)
        r3 = routed[:, :].rearrange("(b p) e -> p b e", p=P)
        CH = 8                               # blocks per load chunk (1.25MB)
        pos = 0
        qi = 0
        while pos < nblk:
            bs = min(CH, nblk - pos)
            eng = nc.sync if (qi % 2 == 0) else nc.scalar
            eng.dma_start(gqv3[:, pos:pos + bs, :], r3[:, pos:pos + bs, :])
            pos += bs
            qi += 1

        for g in range(NT // G):
            r0 = g * G * 128
            memk_t = mpool.tile([P, G * 128], f32, tag="mk")
            nc.sync.dma_start(
                memk_t[:].rearrange("p (a d) -> p a d", d=128),
                mem_k[r0:r0 + G * 128, :].rearrange("(a p) d -> p a d", p=P),
            )
            memv_t = mpool.tile([P, G * 128], f32, tag="mv")
            nc.sync.dma_start(
                memv_t[:].rearrange("p (a d) -> p a d", d=128),
                mem_v[r0:r0 + G * 128, :].rearrange("(a p) d -> p a d", p=P),
            )
            outk_t = opool.tile([P, G * 128], f32, tag="ok")
            outv_t = opool.tile([P, G * 128], f32, tag="ov")

            for pg in range(G // PG):
                # PSUM: 4 tiles x one [512] bank each; [q|v|cnt] at i*512+0..257
                ps = pspool.tile([P, PG * 512], f32, tag="ps")
                ps3 = ps[:].rearrange("p (i c) -> p i c", c=512)
                for i in range(PG):
                    t = g * G + pg * PG + i
                    for col, blk, st, sp in incid[t]:
                        oh = wpool.tile([P, 128], mmdt, tag="oh")
                        nc.vector.tensor_scalar(
                            oh[:], iota_t[:],
                            sv_t[:, col:col + 1], w_t[:, blk:blk + 1],
                            op0=Alu.is_equal, op1=Alu.mult,
                        )
                        nc.tensor.matmul(
                            ps[:, i * 512:i * 512 + 257],
                            lhsT=oh[:],
                            rhs=gqv[:, blk * EL:blk * EL + 257],
                            start=st, stop=sp,
                        )
                # epilogue for the 4-tile group
                cnt = ps3[:, :, 256:257]                      # [P, 4, 1]
                ind = spool.tile([P, PG], f32, tag="ind")
                nc.vector.tensor_scalar(ind[:], cnt, 0.0, None, op0=Alu.is_equal)
                den = spool.tile([P, PG], f32, tag="den")
                nc.vector.tensor_tensor(den[:], cnt, ind[:], op=Alu.add)
                rec = spool.tile([P, PG], f32, tag="rec")
                nc.vector.reciprocal(rec[:], den[:])
                rec01 = spool.tile([P, PG], f32, tag="rec01")
                nc.vector.tensor_scalar(rec01[:], rec[:], UPD, None, op0=Alu.mult)

                # upd = psum * rec01 (per-partition scale) on the ACT engine
                upd = upool.tile([P, PG * 256], f32, tag="upd")
                upd3 = upd[:].rearrange("p (i c) -> p i c", c=256)
                for i in range(PG):
                    nc.scalar.activation(
                        upd3[:, i, :], ps3[:, i, 0:256], Act.Copy,
                        scale=rec01[:, i:i + 1],
                    )
                cs = slice(pg * PG * 128, (pg + 1) * PG * 128)
                ck = upd3[:, :, 0:128]
                cv = upd3[:, :, 128:256]
                ok3 = outk_t[:, cs].rearrange("p (i d) -> p i d", d=128)
                ov3 = outv_t[:, cs].rearrange("p (i d) -> p i d", d=128)
                mk3 = memk_t[:, cs].rearrange("p (i d) -> p i d", d=128)
                mv3 = memv_t[:, cs].rearrange("p (i d) -> p i d", d=128)
                nc.vector.tensor_tensor(ok3, ck, mk3, op=Alu.add)
                nc.gpsimd.tensor_tensor(ov3, cv, mv3, op=Alu.add)

            nc.scalar.dma_start(
                out_k[r0:r0 + G * 128, :].rearrange("(a p) d -> p a d", p=P),
                outk_t[:].rearrange("p (a d) -> p a d", d=128),
            )
            nc.scalar.dma_start(
                out_v[r0:r0 + G * 128, :].rearrange("(a p) d -> p a d", p=P),
                outv_t[:].rearrange("p (a d) -> p a d", d=128),
            )

    nc.compile()
    return nc


def prepare_inputs(inputs):
    """Host-side routing (the all-to-all stand-in): bucket contributions by
    (core, slot-tile) and materialize each core's routed row buffer."""
    mk = np.ascontiguousarray(np.asarray(inputs["memory_keys"], dtype=np.float32))
    mv = np.ascontiguousarray(np.asarray(inputs["memory_values"], dtype=np.float32))
    q = np.asarray(inputs["write_query"], dtype=np.float32)
    v = np.asarray(inputs["write_value"], dtype=np.float32)
    gate = np.asarray(inputs["gate_weights"], dtype=np.float32)
    ti = np.asarray(inputs["top_indices"]).astype(np.int64).reshape(-1)

    qv = np.zeros((B, EL), dtype=np.float32)
    qv[:, 0:DIM] = q
    qv[:, DIM:2 * DIM] = v
    qv[:, 2 * DIM] = 1.0

    a = np.arange(B * K, dtype=np.int64) // K
    key = ti >> 7                       # global 128-slot tile id [0, 512)
    order = np.argsort(key, kind="stable")
    ks = key[order]
    a_s = a[order]
    s_s = (ti & 127)[order].astype(np.float32)
    cnt = np.bincount(key, minlength=NCORES * NT)
    starts = np.zeros(NCORES * NT + 1, dtype=np.int64)
    starts[1:] = np.cumsum(cnt)

    # Shared program structure across cores: per tile, ceil(max-count/128)
    # 128-row blocks.
    cnt2 = cnt.reshape(NCORES, NT)
    cnt_max = cnt2.max(axis=0)
    nfrag = np.maximum(1, -(-cnt_max // 128)).astype(np.int64)
    blk_of = np.zeros(NT + 1, dtype=np.int64)
    blk_of[1:] = np.cumsum(nfrag)
    nblk = int(blk_of[-1])
    incid = tuple(
        tuple(
            (int(blk_of[t]) + i, int(blk_of[t]) + i, i == 0, i == int(nfrag[t]) - 1)
            for i in range(int(nfrag[t]))
        )
        for t in range(NT)
    )

    NI = nblk * 128
    in_maps = []
    for c in range(NCORES):
        routed = np.zeros((NI, EL), dtype=np.float32)
        sv_core = np.full((P, nblk), -1.0, dtype=np.float32)
        wb_core = np.zeros((P, nblk), dtype=np.float32)
        # vectorized fill: destination row for sorted contribution r of tile t
        # is blk_of[t]*128 + (r - starts[c*NT+t])
        lo = int(starts[c * NT])
        hi = int(starts[(c + 1) * NT])
        rows = np.arange(lo, hi)
        t_loc = ks[rows] - c * NT
        pos = rows - starts[ks[rows]]
        dest = blk_of[t_loc] * 128 + pos
        routed[dest] = qv[a_s[rows]]
        sv_core[dest % 128, dest // 128] = s_s[rows]
        wb_core[dest % 128, dest // 128] = gate[a_s[rows]]
        in_maps.append({
            "mem_k": mk[c * SPC:(c + 1) * SPC],
            "mem_v": mv[c * SPC:(c + 1) * SPC],
            "routed": routed,
            "sv": np.ascontiguousarray(sv_core),
            "wb": np.ascontiguousarray(wb_core),
        })
    return in_maps, nblk, incid


def kernel(**inputs):
    from concourse.bass_utils import run_bass_kernel_spmd

    in_maps, nblk, incid = prepare_inputs(inputs)
    bkey = (nblk, incid)
    if bkey not in _BUILD_CACHE:
        _BUILD_CACHE[bkey] = build_nc(nblk, incid)
    nc = _BUILD_CACHE[bkey]

    res = run_bass_kernel_spmd(nc, in_maps, core_ids=list(range(NCORES)))
    out_k = np.concatenate([res.results[c]["out_k"] for c in range(NCORES)], axis=0)
    out_v = np.concatenate([res.results[c]["out_v"] for c in range(NCORES)], axis=0)

    km = np.asarray(inputs["key_momentum"], dtype=np.float32)
    vm = np.asarray(inputs["value_momentum"], dtype=np.float32)
    # mom is zeros in this problem; fall back to a host-side add if it isn't
    if np.any(km):
        out_k = out_k + np.float32(MOMENTUM) * km
    if np.any(vm):
        out_v = out_v + np.float32(MOMENTUM) * vm
    return out_k, out_v
